# revision 20
# baseline (speedup 1.0000x reference)
"""Trainium2 Bass kernel for nn_ChannelSegment (differential-attention MoE).

Sharding: 8 cores = 4 channels x 2 batches; core i handles (b, n) = (i//4, i%4).
Each core runs the full per-channel forward for one [L=1024, CW=512] slice.

Layout strategy: activations kept transposed [feature, token] so per-feature
constants (biases, wq/wk/wh/wn) are per-partition scalars. Per-token scalars
(softmax denominators, rms scales) are broadcast across partitions with tiny
PE matmuls against constant 0/1 matrices. Attention scores are computed
directly in [m, l] (key-major) form, so softmax sums become matmul
reductions through an appended ones-column on V, and no transposes of the
probability matrices are needed.

Matmul dtype: float32r (TF32-like, ~1e-4 relerr, full PE rate at N>=256) for
everything except the probability @ V matmuls which run in bf16.
"""
import os
import sys

sys.path.insert(0, "/opt/trn_rl_repo")

import numpy as np
import ml_dtypes

from concourse import bacc
import concourse.tile as tile
from concourse import mybir
from concourse.bass_utils import run_bass_kernel_spmd

N_CH, CW, H, D, D2 = 4, 512, 8, 64, 32
L, B = 1024, 2
EPS = 1e-6
LAM0 = 0.2
SCALE = float(1.0 / np.sqrt(np.float32(D2)))

F32 = mybir.dt.float32
F32R = mybir.dt.float32r
BF16 = mybir.dt.bfloat16
AF = mybir.ActivationFunctionType
OP = mybir.AluOpType

_cache = {}


def _build():
    from contextlib import ExitStack

    nc = bacc.Bacc("TRN2", target_bir_lowering=False, num_devices=8)

    dp = nc.declare_dram_parameter
    hT_d = dp("hT", [CW, L], F32R, isOutput=False)
    wqk_d = dp("wqk", [CW, 2 * CW], F32R, isOutput=False)
    wv_d = dp("wv", [CW, CW], F32R, isOutput=False)
    wout_d = dp("wout", [CW, CW], F32R, isOutput=False)
    bqk_d = dp("bqk", [8, 128, 1], F32, isOutput=False)
    bv_d = dp("bv", [CW], F32, isOutput=False)
    bout_d = dp("bout", [4, 128, 1], F32, isOutput=False)
    qmul_d = dp("qmul", [4, 128, 1], F32, isOutput=False)
    whs_d = dp("whs", [128, 1], F32, isOutput=False)
    wnw_d = dp("wnw", [4, 128, 1], F32, isOutput=False)
    wrt_d = dp("wrt", [128, 1], F32, isOutput=False)
    lam_d = dp("lam", [1, 1], F32, isOutput=False)
    tri_d = dp("tri", [128, 128], BF16, isOutput=False)
    b4_d = dp("b4", [4, 128], F32R, isOutput=False)
    b2c_d = dp("b2c", [33, 128], F32R, isOutput=False)
    e4_d = dp("e4", [128, 4], F32R, isOutput=False)
    ones_d = dp("ones", [128, 1], F32R, isOutput=False)
    o164_d = dp("o164", [1, 64], F32R, isOutput=False)
    o1128_d = dp("o1128", [1, 128], F32R, isOutput=False)
    yT_d = dp("yT", [CW, L], F32, isOutput=True)
    debug = bool(os.environ.get("KERNEL_DEBUG"))
    if debug:
        dbg_qk_d = dp("dbg_qk", [2 * CW, L], F32, isOutput=True)
        dbg_diffn_d = dp("dbg_diffn", [CW, L], F32, isOutput=True)
        dbg_attn_d = dp("dbg_attn", [CW, L], F32, isOutput=True)

    with tile.TileContext(nc) as tc:
        est = ExitStack()
        est.enter_context(nc.allow_low_precision(reason="float32r intermediates are 4-byte"))
        persist = est.enter_context(tc.tile_pool(name="persist", bufs=1))
        ps_mm = est.enter_context(tc.tile_pool(name="ps_mm", bufs=3, space="PSUM"))
        ps_acc = est.enter_context(tc.tile_pool(name="ps_acc", bufs=4, space="PSUM"))
        ps_sm = est.enter_context(tc.tile_pool(name="ps_sm", bufs=1, space="PSUM"))
        p_pool = est.enter_context(tc.tile_pool(name="p_pool", bufs=3))
        sqp = est.enter_context(tc.tile_pool(name="sqp", bufs=2))
        osbp = est.enter_context(tc.tile_pool(name="osbp", bufs=4))
        up = est.enter_context(tc.tile_pool(name="up", bufs=2))
        usqp = est.enter_context(tc.tile_pool(name="usqp", bufs=2))
        rowp = est.enter_context(tc.tile_pool(name="rowp", bufs=2))
        tmpp = est.enter_context(tc.tile_pool(name="tmpp", bufs=2))
        yp = est.enter_context(tc.tile_pool(name="yp", bufs=2))
        vsp = est.enter_context(tc.tile_pool(name="vsp", bufs=2))
        rowbig = est.enter_context(tc.tile_pool(name="rowbig", bufs=2))

        dma = nc.sync.dma_start

        # ---- load constants / inputs ----
        hT = [persist.tile([128, L], F32R, tag=f"hT{k}", name=f"hT{k}") for k in range(4)]
        for k in range(4):
            dma(out=hT[k], in_=hT_d[128 * k : 128 * (k + 1), :])
        wqk = [persist.tile([128, 2 * CW], F32R, tag=f"wq{k}", name=f"wq{k}") for k in range(4)]
        for k in range(4):
            dma(out=wqk[k], in_=wqk_d[128 * k : 128 * (k + 1), :])
        wv = [persist.tile([128, CW], F32R, tag=f"wv{k}", name=f"wv{k}") for k in range(4)]
        for k in range(4):
            dma(out=wv[k], in_=wv_d[128 * k : 128 * (k + 1), :])
        wout_sb = [persist.tile([128, CW], F32R, tag=f"wo{k}", name=f"wo{k}") for k in range(4)]
        for k in range(4):
            dma(out=wout_sb[k], in_=wout_d[128 * k : 128 * (k + 1), :])

        bqk = [persist.tile([128, 1], F32, tag=f"bqk{j}", name=f"bqk{j}") for j in range(8)]
        for j in range(8):
            dma(out=bqk[j], in_=bqk_d[j])
        bv_bc = persist.tile([128, CW], F32, tag="bv_bc", name="bv_bc")
        dma(out=bv_bc, in_=bv_d[:].partition_broadcast(128))
        bout = [persist.tile([128, 1], F32, tag=f"bout{j}", name=f"bout{j}") for j in range(4)]
        for j in range(4):
            dma(out=bout[j], in_=bout_d[j])
        qmul = [persist.tile([128, 1], F32, tag=f"qmul{j}", name=f"qmul{j}") for j in range(4)]
        for j in range(4):
            dma(out=qmul[j], in_=qmul_d[j])
        whs = persist.tile([128, 1], F32, tag="whs", name="whs")
        dma(out=whs, in_=whs_d[:])
        wnw = [persist.tile([128, 1], F32, tag=f"wnw{j}", name=f"wnw{j}") for j in range(4)]
        for j in range(4):
            dma(out=wnw[j], in_=wnw_d[j])
        wrt = persist.tile([128, 1], F32, tag="wrt", name="wrt")
        dma(out=wrt, in_=wrt_d[:])
        lam = persist.tile([1, 1], F32, tag="lam", name="lam")
        dma(out=lam, in_=lam_d[:])
        tri = persist.tile([128, 128], BF16, tag="tri", name="tri")
        dma(out=tri, in_=tri_d[:])
        e4 = persist.tile([128, 4], F32R, tag="e4", name="e4")
        dma(out=e4, in_=e4_d[:])
        b4 = persist.tile([4, 128], F32R, tag="b4", name="b4")
        dma(out=b4, in_=b4_d[:])
        b2c = persist.tile([33, 128], F32R, tag="b2c", name="b2c")
        dma(out=b2c, in_=b2c_d[:])
        ones = persist.tile([128, 1], F32R, tag="ones", name="ones")
        dma(out=ones, in_=ones_d[:])
        o164 = persist.tile([1, 64], F32R, tag="o164", name="o164")
        dma(out=o164, in_=o164_d[:])
        o1128 = persist.tile([1, 128], F32R, tag="o1128", name="o1128")
        dma(out=o1128, in_=o1128_d[:])
        eps_sb = persist.tile([128, 1], F32, tag="eps_sb", name="eps_sb")
        nc.vector.memset(eps_sb, EPS)

        # ---- MM1a: qkT [1024, L] = silu(wqk.T @ hT + bqk) ----
        qkT = [persist.tile([128, L], F32R, tag=f"qkT{j}", name=f"qkT{j}") for j in range(8)]
        for j in range(8):
            for c in range(2):
                ps = ps_mm.tile([128, 512], F32, tag="mm", name="mm")
                for k in range(4):
                    nc.tensor.matmul(
                        ps,
                        wqk[k][:, 128 * j : 128 * (j + 1)],
                        hT[k][:, 512 * c : 512 * (c + 1)],
                        start=(k == 0),
                        stop=(k == 3),
                    )
                nc.scalar.activation(
                    out=qkT[j][:, 512 * c : 512 * (c + 1)], in_=ps,
                    func=AF.Silu, bias=bqk[j],
                )

        # ---- MM1b: v = silu(h @ wv + bv), packed into v_aug with ones col ----
        v_aug = [persist.tile([128, 8, 65], BF16, tag=f"vaug{t}", name=f"vaug{t}") for t in range(8)]
        for t in range(8):
            nc.vector.memset(v_aug[t][:, :, 64:65], 1.0)
            ps = ps_mm.tile([128, 512], F32, tag="mm", name="mm")
            for k in range(4):
                nc.tensor.matmul(
                    ps,
                    hT[k][:, 128 * t : 128 * (t + 1)],
                    wv[k],
                    start=(k == 0),
                    stop=(k == 3),
                )
            t1 = vsp.tile([128, 512], F32, tag="vscratch", name="vscratch")
            nc.vector.tensor_add(out=t1, in0=ps, in1=bv_bc)
            nc.scalar.activation(
                out=v_aug[t][:, :, 0:64],
                in_=t1.rearrange("p (h d) -> p h d", d=64),
                func=AF.Silu,
            )

        # ---- RMS of q1/q2/k1/k2 groups; fold wq*wk into q ----
        # squares on gpsimd, group-mean via matmul, rsqrt via one batched
        # Abs_reciprocal_sqrt per tile, broadcast back via PE matmul
        for j in range(8):
            msqs_j = rowbig.tile([4, L], F32, tag="msqs", name="msqs")
            rall_j = rowbig.tile([4, L], F32R, tag="rall", name="rall")
            for c in range(2):
                sq = sqp.tile([128, 512], F32R, tag="sq", name="sq")
                nc.gpsimd.tensor_mul(
                    out=sq,
                    in0=qkT[j][:, 512 * c : 512 * (c + 1)],
                    in1=qkT[j][:, 512 * c : 512 * (c + 1)],
                )
                msq = ps_sm.tile([4, 512], F32, tag="sm", name="sm")
                nc.tensor.matmul(msq, e4, sq, start=True, stop=True)
                nc.vector.tensor_copy(out=msqs_j[:, 512 * c : 512 * (c + 1)], in_=msq)
            nc.scalar.activation(
                out=rall_j, in_=msqs_j, func=AF.Abs_reciprocal_sqrt,
                scale=1.0 / 32.0, bias=eps_sb[0:4, :],
            )
            for c in range(2):
                rbc = ps_mm.tile([128, 512], F32, tag="mm", name="mm")
                nc.tensor.matmul(
                    rbc, b4, rall_j[:, 512 * c : 512 * (c + 1)],
                    start=True, stop=True,
                )
                if j < 4:
                    nc.vector.scalar_tensor_tensor(
                        out=qkT[j][:, 512 * c : 512 * (c + 1)],
                        in0=qkT[j][:, 512 * c : 512 * (c + 1)],
                        scalar=qmul[j],
                        in1=rbc,
                        op0=OP.mult,
                        op1=OP.mult,
                    )
                else:
                    nc.vector.tensor_mul(
                        out=qkT[j][:, 512 * c : 512 * (c + 1)],
                        in0=qkT[j][:, 512 * c : 512 * (c + 1)],
                        in1=rbc,
                    )

        # ---- attention per head ----
        diffn = [persist.tile([128, L], F32R, tag=f"diffn{j}", name=f"diffn{j}") for j in range(4)]
        msq2 = [persist.tile([33, L], F32, tag=f"msq2{j}", name=f"msq2{j}") for j in range(4)]
        for j in range(4):
            nc.vector.memset(msq2[j], 1.0)
        for h in range(H):
            jq = h // 2
            jk = 4 + h // 2
            po = 64 * (h % 2)
            for c in range(2):
                out_ps = []
                for br in range(2):
                    out_ps.append(ps_acc.tile([65, 512], F32, tag="acc", name="acc"))
                for t in range(4 * c + 4):
                    off = max(0, 128 * (t - 4 * c))
                    w = 512 - off
                    ps_b = []
                    for br in range(2):
                        bo = po + 32 * br
                        s_ps = ps_mm.tile([128, 512], F32, tag="mm", name="mm")
                        nc.tensor.matmul(
                            s_ps[:, 0:w],
                            qkT[jk][bo : bo + 32, 128 * t : 128 * (t + 1)],
                            qkT[jq][bo : bo + 32, 512 * c + off : 512 * (c + 1)],
                            start=True,
                            stop=True,
                            tile_position=(bo, 0),
                        )
                        ps_b.append(s_ps)
                    for br in range(2):
                        p = p_pool.tile([128, 512], BF16, tag="p", name="p")
                        nc.scalar.activation(
                            out=p[:, 0:w], in_=ps_b[br][:, 0:w], func=AF.Exp, scale=SCALE
                        )
                        if t >= 4 * c:
                            nc.gpsimd.tensor_mul(
                                out=p[:, 0:128], in0=p[:, 0:128], in1=tri
                            )
                        nc.tensor.matmul(
                            out_ps[br][:, off:512],
                            v_aug[t][:, h, :],
                            p[:, 0:w],
                            start=(t == 0),
                            stop=(t == 4 * c + 3),
                        )
                # drain accumulators, compute u = o1*den2 - o2*(lam*den1)
                osb1 = osbp.tile([65, 512], F32, tag="osb", name="osb")
                osb2 = osbp.tile([65, 512], F32, tag="osb", name="osb")
                nc.vector.tensor_copy(out=osb1, in_=out_ps[0])
                nc.vector.tensor_copy(out=osb2, in_=out_ps[1])
                ld1 = rowp.tile([1, 512], F32R, tag="row", name="row")
                nc.vector.tensor_scalar_mul(out=ld1, in0=osb1[64:65, :], scalar1=lam)
                den2r = rowp.tile([1, 512], F32R, tag="row", name="row")
                nc.vector.tensor_copy(out=den2r, in_=osb2[64:65, :])
                b1 = ps_sm.tile([64, 512], F32, tag="sm", name="sm")
                nc.tensor.matmul(b1, o164, ld1, start=True, stop=True)
                m1 = up.tile([64, 512], F32, tag="u", name="u")
                nc.vector.tensor_mul(out=m1, in0=osb2[0:64, :], in1=b1)
                b2 = ps_sm.tile([64, 512], F32, tag="sm", name="sm")
                nc.tensor.matmul(b2, o164, den2r, start=True, stop=True)
                u = up.tile([64, 512], F32, tag="u", name="u")
                nc.vector.tensor_mul(out=u, in0=osb1[0:64, :], in1=b2)
                nc.vector.tensor_sub(out=u, in0=u, in1=m1)
                usq = usqp.tile([64, 512], F32R, tag="usq", name="usq")
                nc.gpsimd.tensor_mul(out=usq, in0=u, in1=u)
                r32 = 32 * (h % 2)
                nc.gpsimd.tensor_reduce(
                    out=msq2[jq][r32 : r32 + 1, 512 * c : 512 * (c + 1)],
                    in_=usq,
                    axis=mybir.AxisListType.C,
                    op=OP.add,
                )
                # diffn slice = u * whs (rt scale applied later, batched)
                nc.vector.tensor_scalar_mul(
                    out=diffn[jq][po : po + 64, 512 * c : 512 * (c + 1)],
                    in0=u,
                    scalar1=whs[0:64, :],
                )

        # ---- batched rsqrt for the diff rms + apply ----
        for j in range(4):
            rt2 = rowbig.tile([33, L], F32R, tag="rall", name="rt2")
            nc.scalar.activation(
                out=rt2, in_=msq2[j], func=AF.Abs_reciprocal_sqrt,
                scale=1.0 / 64.0, bias=eps_sb[0:33, :],
            )
            for c in range(2):
                rtb = ps_mm.tile([128, 512], F32, tag="mm", name="mm")
                nc.tensor.matmul(
                    rtb, b2c, rt2[:, 512 * c : 512 * (c + 1)],
                    start=True, stop=True,
                )
                nc.vector.tensor_mul(
                    out=diffn[j][:, 512 * c : 512 * (c + 1)],
                    in0=diffn[j][:, 512 * c : 512 * (c + 1)],
                    in1=rtb,
                )

        if debug:
            for j in range(8):
                dma(out=dbg_qk_d[128 * j : 128 * (j + 1), :], in_=qkT[j].bitcast(F32))
            for j in range(4):
                dma(out=dbg_diffn_d[128 * j : 128 * (j + 1), :], in_=diffn[j].bitcast(F32))

        # ---- MM2: attn_outT = silu(wout.T @ diffn + bout) ----
        attn = [persist.tile([128, L], F32R, tag=f"attn{j}", name=f"attn{j}") for j in range(4)]
        for j in range(4):
            for c in range(2):
                ps = ps_mm.tile([128, 512], F32, tag="mm", name="mm")
                for k in range(4):
                    nc.tensor.matmul(
                        ps,
                        wout_sb[k][:, 128 * j : 128 * (j + 1)],
                        diffn[k][:, 512 * c : 512 * (c + 1)],
                        start=(k == 0),
                        stop=(k == 3),
                    )
                nc.scalar.activation(
                    out=attn[j][:, 512 * c : 512 * (c + 1)], in_=ps,
                    func=AF.Silu, bias=bout[j],
                )

        if debug:
            for j in range(4):
                dma(out=dbg_attn_d[128 * j : 128 * (j + 1), :], in_=attn[j].bitcast(F32))

        # ---- final rms over CW=512 + residual + routing weight ----
        msqf = persist.tile([1, L], F32, tag="msqf", name="msqf")
        for c in range(2):
            fin = ps_sm.tile([1, 512], F32, tag="sm", name="sm")
            for j in range(4):
                asq = sqp.tile([128, 512], F32R, tag="sq", name="sq")
                nc.gpsimd.tensor_mul(
                    out=asq,
                    in0=attn[j][:, 512 * c : 512 * (c + 1)],
                    in1=attn[j][:, 512 * c : 512 * (c + 1)],
                )
                nc.tensor.matmul(fin, ones, asq, start=(j == 0), stop=(j == 3))
            nc.vector.tensor_copy(out=msqf[:, 512 * c : 512 * (c + 1)], in_=fin)
        rf = persist.tile([1, L], F32R, tag="rf", name="rf")
        nc.scalar.activation(
            out=rf, in_=msqf, func=AF.Abs_reciprocal_sqrt,
            scale=1.0 / 512.0, bias=eps_sb[0:1, :],
        )
        for j in range(4):
            nc.vector.tensor_scalar_mul(out=hT[j], in0=hT[j], scalar1=wrt)
        for c in range(2):
            rfbc = ps_mm.tile([128, 512], F32, tag="mm", name="mm")
            nc.tensor.matmul(
                rfbc, o1128, rf[:, 512 * c : 512 * (c + 1)],
                start=True, stop=True,
            )
            for j in range(4):
                tmp = tmpp.tile([128, 512], F32, tag="tmp", name="tmp")
                nc.vector.tensor_mul(
                    out=tmp, in0=attn[j][:, 512 * c : 512 * (c + 1)], in1=rfbc
                )
                y = yp.tile([128, 512], F32, tag="y", name="y")
                nc.vector.scalar_tensor_tensor(
                    out=y,
                    in0=tmp,
                    scalar=wnw[j],
                    in1=hT[j][:, 512 * c : 512 * (c + 1)],
                    op0=OP.mult,
                    op1=OP.add,
                )
                dma(out=yT_d[128 * j : 128 * (j + 1), 512 * c : 512 * (c + 1)], in_=y)
        est.close()

    nc.compile()
    return nc


def kernel(x, routing_weights, Wqkv, bqkv, Wout, bout, lq1, lk1, lq2, lk2, wq, wk, wh, wn):
    if "nc" not in _cache:
        _cache["nc"] = _build()
    nc = _cache["nc"]

    x = np.asarray(x, np.float32)
    routing_weights = np.asarray(routing_weights, np.float32)

    tri = np.triu(np.ones((128, 128), np.float32)).astype(ml_dtypes.bfloat16)
    e4 = np.zeros((128, 4), np.float32)
    for g in range(4):
        e4[32 * g : 32 * (g + 1), g] = 1.0
    b4 = e4.T.copy()
    b2c = np.zeros((33, 128), np.float32)
    b2c[0, 0:64] = 1.0
    b2c[32, 64:128] = 1.0
    ones = np.ones((128, 1), np.float32)
    o164 = np.ones((1, 64), np.float32)
    o1128 = np.ones((1, 128), np.float32)

    in_maps = []
    for i in range(8):
        b, n = i // 4, i % 4
        w = float(routing_weights[b, n])
        lam = float(
            np.exp(np.dot(lq1[n], lk1[n]).astype(np.float32))
            - np.exp(np.dot(lq2[n], lk2[n]).astype(np.float32))
            + np.float32(LAM0)
        )
        wqwk = (wq[n] * wk[n]).astype(np.float32)  # [32]
        in_maps.append(
            dict(
                hT=np.ascontiguousarray(x[b, :, CW * n : CW * (n + 1)].T),
                wqk=np.ascontiguousarray(Wqkv[n][:, : 2 * CW]),
                wv=np.ascontiguousarray(Wqkv[n][:, 2 * CW :]),
                wout=np.ascontiguousarray(Wout[n]),
                bqk=np.ascontiguousarray(bqkv[n][: 2 * CW].reshape(8, 128, 1)),
                bv=np.ascontiguousarray(bqkv[n][2 * CW :]),
                bout=np.ascontiguousarray(bout[n].reshape(4, 128, 1)),
                qmul=np.ascontiguousarray(np.tile(wqwk, 16).reshape(4, 128, 1)),
                whs=np.ascontiguousarray((np.tile(wh[n], 2) * 0.8).reshape(128, 1)).astype(np.float32),
                wnw=np.ascontiguousarray((wn[n] * w).reshape(4, 128, 1)).astype(np.float32),
                wrt=np.full((128, 1), w, np.float32),
                lam=np.full((1, 1), lam, np.float32),
                tri=tri,
                e4=e4,
                b4=b4,
                b2c=b2c,
                ones=ones,
                o164=o164,
                o1128=o1128,
            )
        )

    prof_dir = os.environ.get("KERNEL_PROFILE_DIR")
    if prof_dir:
        res = run_bass_kernel_spmd(
            nc, in_maps, list(range(8)), trace=True, tmpdir=prof_dir
        )
        _cache["exec_time_ns"] = res.exec_time_ns
    else:
        res = run_bass_kernel_spmd(nc, in_maps, list(range(8)))

    out = np.empty((B, L, N_CH * CW), np.float32)
    for i in range(8):
        b, n = i // 4, i % 4
        out[b, :, CW * n : CW * (n + 1)] = res.results[i]["yT"].T
    return out


# revision 21
# speedup vs baseline: 3.9084x; 3.9084x over previous
"""Trainium2 Bass kernel for nn_ChannelSegment (differential-attention MoE).

Sharding: 8 cores = 4 channels x 2 batches; core i handles (b, n) = (i//4, i%4).
Each core runs the full per-channel forward for one [L=1024, CW=512] slice.

Layout strategy: activations kept transposed [feature, token] so per-feature
constants (biases, wq/wk/wh/wn) are per-partition scalars. Per-token scalars
(softmax denominators, rms scales) are broadcast across partitions with tiny
PE matmuls against constant 0/1 matrices. Attention scores are computed
directly in [m, l] (key-major) form, so softmax sums become matmul
reductions through an appended ones-column on V, and no transposes of the
probability matrices are needed.

Matmul dtype: float32r (TF32-like, ~1e-4 relerr, full PE rate at N>=256) for
everything except the probability @ V matmuls which run in bf16.
"""
import os
import sys

sys.path.insert(0, "/opt/trn_rl_repo")

import numpy as np
import ml_dtypes

from concourse import bacc
import concourse.tile as tile
from concourse import mybir
from concourse.bass_utils import run_bass_kernel_spmd

N_CH, CW, H, D, D2 = 4, 512, 8, 64, 32
L, B = 1024, 2
EPS = 1e-6
LAM0 = 0.2
SCALE = float(1.0 / np.sqrt(np.float32(D2)))

F32 = mybir.dt.float32
F32R = mybir.dt.float32r
BF16 = mybir.dt.bfloat16
AF = mybir.ActivationFunctionType
OP = mybir.AluOpType

_cache = {}


def _build():
    from contextlib import ExitStack

    nc = bacc.Bacc("TRN2", target_bir_lowering=False, num_devices=8)

    dp = nc.declare_dram_parameter
    hT_d = dp("hT", [CW, L], F32R, isOutput=False)
    wqk_d = dp("wqk", [CW, 2 * CW], F32R, isOutput=False)
    wv_d = dp("wv", [CW, CW], F32R, isOutput=False)
    wout_d = dp("wout", [CW, CW], F32R, isOutput=False)
    bqk_d = dp("bqk", [8, 128, 1], F32, isOutput=False)
    bv_d = dp("bv", [CW], F32, isOutput=False)
    bout_d = dp("bout", [4, 128, 1], F32, isOutput=False)
    qmul_d = dp("qmul", [4, 128, 1], F32, isOutput=False)
    whs_d = dp("whs", [128, 1], F32, isOutput=False)
    wnw_d = dp("wnw", [4, 128, 1], F32, isOutput=False)
    wrt_d = dp("wrt", [128, 1], F32, isOutput=False)
    lam_d = dp("lam", [1, 1], F32, isOutput=False)
    tri_d = dp("tri", [128, 128], BF16, isOutput=False)
    b4_d = dp("b4", [4, 128], F32R, isOutput=False)
    b2c_d = dp("b2c", [33, 128], F32R, isOutput=False)
    e4_d = dp("e4", [128, 4], F32R, isOutput=False)
    ones_d = dp("ones", [128, 1], F32R, isOutput=False)
    o164_d = dp("o164", [1, 64], F32R, isOutput=False)
    o1128_d = dp("o1128", [1, 128], F32R, isOutput=False)
    yT_d = dp("yT", [CW, L], F32, isOutput=True)
    debug = bool(os.environ.get("KERNEL_DEBUG"))
    if debug:
        dbg_qk_d = dp("dbg_qk", [2 * CW, L], F32, isOutput=True)
        dbg_diffn_d = dp("dbg_diffn", [CW, L], F32, isOutput=True)
        dbg_attn_d = dp("dbg_attn", [CW, L], F32, isOutput=True)

    with tile.TileContext(nc) as tc:
        est = ExitStack()
        est.enter_context(nc.allow_low_precision(reason="float32r intermediates are 4-byte"))
        persist = est.enter_context(tc.tile_pool(name="persist", bufs=1))
        ps_mm = est.enter_context(tc.tile_pool(name="ps_mm", bufs=3, space="PSUM"))
        ps_acc = est.enter_context(tc.tile_pool(name="ps_acc", bufs=4, space="PSUM"))
        ps_sm = est.enter_context(tc.tile_pool(name="ps_sm", bufs=1, space="PSUM"))
        p_pool = est.enter_context(tc.tile_pool(name="p_pool", bufs=3))
        sqp = est.enter_context(tc.tile_pool(name="sqp", bufs=2))
        osbp = est.enter_context(tc.tile_pool(name="osbp", bufs=4))
        up = est.enter_context(tc.tile_pool(name="up", bufs=2))
        usqp = est.enter_context(tc.tile_pool(name="usqp", bufs=2))
        rowp = est.enter_context(tc.tile_pool(name="rowp", bufs=2))
        tmpp = est.enter_context(tc.tile_pool(name="tmpp", bufs=2))
        yp = est.enter_context(tc.tile_pool(name="yp", bufs=2))
        vsp = est.enter_context(tc.tile_pool(name="vsp", bufs=2))
        rowbig = est.enter_context(tc.tile_pool(name="rowbig", bufs=2))

        dma = nc.sync.dma_start

        # ---- load constants / inputs ----
        hT = [persist.tile([128, L], F32R, tag=f"hT{k}", name=f"hT{k}") for k in range(4)]
        for k in range(4):
            dma(out=hT[k], in_=hT_d[128 * k : 128 * (k + 1), :])
        wqk = [persist.tile([128, 2 * CW], F32R, tag=f"wq{k}", name=f"wq{k}") for k in range(4)]
        for k in range(4):
            dma(out=wqk[k], in_=wqk_d[128 * k : 128 * (k + 1), :])
        wv = [persist.tile([128, CW], F32R, tag=f"wv{k}", name=f"wv{k}") for k in range(4)]
        for k in range(4):
            dma(out=wv[k], in_=wv_d[128 * k : 128 * (k + 1), :])
        wout_sb = [persist.tile([128, CW], F32R, tag=f"wo{k}", name=f"wo{k}") for k in range(4)]
        for k in range(4):
            dma(out=wout_sb[k], in_=wout_d[128 * k : 128 * (k + 1), :])

        bqk = [persist.tile([128, 1], F32, tag=f"bqk{j}", name=f"bqk{j}") for j in range(8)]
        for j in range(8):
            dma(out=bqk[j], in_=bqk_d[j])
        bv_bc = persist.tile([128, CW], F32, tag="bv_bc", name="bv_bc")
        dma(out=bv_bc, in_=bv_d[:].partition_broadcast(128))
        bout = [persist.tile([128, 1], F32, tag=f"bout{j}", name=f"bout{j}") for j in range(4)]
        for j in range(4):
            dma(out=bout[j], in_=bout_d[j])
        qmul = [persist.tile([128, 1], F32, tag=f"qmul{j}", name=f"qmul{j}") for j in range(4)]
        for j in range(4):
            dma(out=qmul[j], in_=qmul_d[j])
        whs = persist.tile([128, 1], F32, tag="whs", name="whs")
        dma(out=whs, in_=whs_d[:])
        wnw = [persist.tile([128, 1], F32, tag=f"wnw{j}", name=f"wnw{j}") for j in range(4)]
        for j in range(4):
            dma(out=wnw[j], in_=wnw_d[j])
        wrt = persist.tile([128, 1], F32, tag="wrt", name="wrt")
        dma(out=wrt, in_=wrt_d[:])
        lam = persist.tile([1, 1], F32, tag="lam", name="lam")
        dma(out=lam, in_=lam_d[:])
        tri = persist.tile([128, 128], BF16, tag="tri", name="tri")
        dma(out=tri, in_=tri_d[:])
        e4 = persist.tile([128, 4], F32R, tag="e4", name="e4")
        dma(out=e4, in_=e4_d[:])
        b4 = persist.tile([4, 128], F32R, tag="b4", name="b4")
        dma(out=b4, in_=b4_d[:])
        b2c = persist.tile([33, 128], F32R, tag="b2c", name="b2c")
        dma(out=b2c, in_=b2c_d[:])
        ones = persist.tile([128, 1], F32R, tag="ones", name="ones")
        dma(out=ones, in_=ones_d[:])
        o164 = persist.tile([1, 64], F32R, tag="o164", name="o164")
        dma(out=o164, in_=o164_d[:])
        o1128 = persist.tile([1, 128], F32R, tag="o1128", name="o1128")
        dma(out=o1128, in_=o1128_d[:])
        eps_sb = persist.tile([128, 1], F32, tag="eps_sb", name="eps_sb")
        nc.vector.memset(eps_sb, EPS)

        # ---- MM1a: qkT [1024, L] = silu(wqk.T @ hT + bqk) ----
        qkT = [persist.tile([128, L], F32R, tag=f"qkT{j}", name=f"qkT{j}") for j in range(8)]
        for j in range(8):
            for c in range(2):
                ps = ps_mm.tile([128, 512], F32, tag="mm", name="mm")
                for k in range(4):
                    nc.tensor.matmul(
                        ps,
                        wqk[k][:, 128 * j : 128 * (j + 1)],
                        hT[k][:, 512 * c : 512 * (c + 1)],
                        start=(k == 0),
                        stop=(k == 3),
                    )
                nc.scalar.activation(
                    out=qkT[j][:, 512 * c : 512 * (c + 1)], in_=ps,
                    func=AF.Silu, bias=bqk[j],
                )

        # ---- MM1b: v = silu(h @ wv + bv), packed into v_aug with ones col ----
        v_aug = [persist.tile([128, 8, 65], BF16, tag=f"vaug{t}", name=f"vaug{t}") for t in range(8)]
        for t in range(8):
            nc.vector.memset(v_aug[t][:, :, 64:65], 1.0)
            ps = ps_mm.tile([128, 512], F32, tag="mm", name="mm")
            for k in range(4):
                nc.tensor.matmul(
                    ps,
                    hT[k][:, 128 * t : 128 * (t + 1)],
                    wv[k],
                    start=(k == 0),
                    stop=(k == 3),
                )
            t1 = vsp.tile([128, 512], F32, tag="vscratch", name="vscratch")
            nc.vector.tensor_add(out=t1, in0=ps, in1=bv_bc)
            nc.scalar.activation(
                out=v_aug[t][:, :, 0:64],
                in_=t1.rearrange("p (h d) -> p h d", d=64),
                func=AF.Silu,
            )

        # ---- RMS of q1/q2/k1/k2 groups; fold wq*wk into q ----
        # squares on gpsimd, group-mean via matmul, rsqrt via one batched
        # Abs_reciprocal_sqrt per tile, broadcast back via PE matmul
        for j in range(8):
            msqs_j = rowbig.tile([4, L], F32, tag="msqs", name="msqs")
            rall_j = rowbig.tile([4, L], F32R, tag="rall", name="rall")
            for c in range(2):
                sq = sqp.tile([128, 512], F32R, tag="sq", name="sq")
                nc.gpsimd.tensor_mul(
                    out=sq,
                    in0=qkT[j][:, 512 * c : 512 * (c + 1)],
                    in1=qkT[j][:, 512 * c : 512 * (c + 1)],
                )
                msq = ps_sm.tile([4, 512], F32, tag="sm", name="sm")
                nc.tensor.matmul(msq, e4, sq, start=True, stop=True)
                nc.vector.tensor_copy(out=msqs_j[:, 512 * c : 512 * (c + 1)], in_=msq)
            nc.scalar.activation(
                out=rall_j, in_=msqs_j, func=AF.Abs_reciprocal_sqrt,
                scale=1.0 / 32.0, bias=eps_sb[0:4, :],
            )
            for c in range(2):
                rbc = ps_mm.tile([128, 512], F32, tag="mm", name="mm")
                nc.tensor.matmul(
                    rbc, b4, rall_j[:, 512 * c : 512 * (c + 1)],
                    start=True, stop=True,
                )
                if j < 4:
                    nc.vector.scalar_tensor_tensor(
                        out=qkT[j][:, 512 * c : 512 * (c + 1)],
                        in0=qkT[j][:, 512 * c : 512 * (c + 1)],
                        scalar=qmul[j],
                        in1=rbc,
                        op0=OP.mult,
                        op1=OP.mult,
                    )
                else:
                    nc.vector.tensor_mul(
                        out=qkT[j][:, 512 * c : 512 * (c + 1)],
                        in0=qkT[j][:, 512 * c : 512 * (c + 1)],
                        in1=rbc,
                    )

        # ---- attention per head ----
        diffn = [persist.tile([128, L], F32R, tag=f"diffn{j}", name=f"diffn{j}") for j in range(4)]
        msq2 = [persist.tile([33, L], F32, tag=f"msq2{j}", name=f"msq2{j}") for j in range(4)]
        for j in range(4):
            nc.vector.memset(msq2[j], 1.0)
        for h in range(H):
            jq = h // 2
            jk = 4 + h // 2
            po = 64 * (h % 2)
            for c in range(2):
                out_ps = []
                for br in range(2):
                    out_ps.append(ps_acc.tile([65, 512], F32, tag="acc", name="acc"))
                for t in range(4 * c + 4):
                    off = max(0, 128 * (t - 4 * c))
                    w = 512 - off
                    ps_b = []
                    for br in range(2):
                        bo = po + 32 * br
                        s_ps = ps_mm.tile([128, 512], F32, tag="mm", name="mm")
                        nc.tensor.matmul(
                            s_ps[:, 0:w],
                            qkT[jk][bo : bo + 32, 128 * t : 128 * (t + 1)],
                            qkT[jq][bo : bo + 32, 512 * c + off : 512 * (c + 1)],
                            start=True,
                            stop=True,
                            tile_position=(bo, 0),
                        )
                        ps_b.append(s_ps)
                    for br in range(2):
                        p = p_pool.tile([128, 512], BF16, tag="p", name="p")
                        nc.scalar.activation(
                            out=p[:, 0:w], in_=ps_b[br][:, 0:w], func=AF.Exp, scale=SCALE
                        )
                        if t >= 4 * c:
                            nc.gpsimd.tensor_mul(
                                out=p[:, 0:128], in0=p[:, 0:128], in1=tri
                            )
                        nc.tensor.matmul(
                            out_ps[br][:, off:512],
                            v_aug[t][:, h, :],
                            p[:, 0:w],
                            start=(t == 0),
                            stop=(t == 4 * c + 3),
                        )
                # drain accumulators, compute u = o1*den2 - o2*(lam*den1)
                osb1 = osbp.tile([65, 512], F32, tag="osb", name="osb")
                osb2 = osbp.tile([65, 512], F32, tag="osb", name="osb")
                nc.vector.tensor_copy(out=osb1, in_=out_ps[0])
                nc.vector.tensor_copy(out=osb2, in_=out_ps[1])
                ld1 = rowp.tile([1, 512], F32R, tag="row", name="row")
                nc.vector.tensor_scalar_mul(out=ld1, in0=osb1[64:65, :], scalar1=lam)
                den2r = rowp.tile([1, 512], F32R, tag="row", name="row")
                nc.vector.tensor_copy(out=den2r, in_=osb2[64:65, :])
                b1 = ps_sm.tile([64, 512], F32, tag="sm", name="sm")
                nc.tensor.matmul(b1, o164, ld1, start=True, stop=True)
                m1 = up.tile([64, 512], F32, tag="u", name="u")
                nc.vector.tensor_mul(out=m1, in0=osb2[0:64, :], in1=b1)
                b2 = ps_sm.tile([64, 512], F32, tag="sm", name="sm")
                nc.tensor.matmul(b2, o164, den2r, start=True, stop=True)
                u = up.tile([64, 512], F32, tag="u", name="u")
                nc.vector.tensor_mul(out=u, in0=osb1[0:64, :], in1=b2)
                nc.vector.tensor_sub(out=u, in0=u, in1=m1)
                usq = usqp.tile([64, 512], F32R, tag="usq", name="usq")
                nc.gpsimd.tensor_mul(out=usq, in0=u, in1=u)
                dmsq = ps_sm.tile([1, 512], F32, tag="sm", name="sm")
                nc.tensor.matmul(dmsq, ones[0:64, :], usq, start=True, stop=True)
                r32 = 32 * (h % 2)
                nc.vector.tensor_copy(
                    out=msq2[jq][r32 : r32 + 1, 512 * c : 512 * (c + 1)], in_=dmsq
                )
                # diffn slice = u * whs (rt scale applied later, batched)
                nc.vector.tensor_scalar_mul(
                    out=diffn[jq][po : po + 64, 512 * c : 512 * (c + 1)],
                    in0=u,
                    scalar1=whs[0:64, :],
                )

        # ---- batched rsqrt for the diff rms + apply ----
        for j in range(4):
            rt2 = rowbig.tile([33, L], F32R, tag="rall", name="rt2")
            nc.scalar.activation(
                out=rt2, in_=msq2[j], func=AF.Abs_reciprocal_sqrt,
                scale=1.0 / 64.0, bias=eps_sb[0:33, :],
            )
            for c in range(2):
                rtb = ps_mm.tile([128, 512], F32, tag="mm", name="mm")
                nc.tensor.matmul(
                    rtb, b2c, rt2[:, 512 * c : 512 * (c + 1)],
                    start=True, stop=True,
                )
                nc.vector.tensor_mul(
                    out=diffn[j][:, 512 * c : 512 * (c + 1)],
                    in0=diffn[j][:, 512 * c : 512 * (c + 1)],
                    in1=rtb,
                )

        if debug:
            for j in range(8):
                dma(out=dbg_qk_d[128 * j : 128 * (j + 1), :], in_=qkT[j].bitcast(F32))
            for j in range(4):
                dma(out=dbg_diffn_d[128 * j : 128 * (j + 1), :], in_=diffn[j].bitcast(F32))

        # ---- MM2: attn_outT = silu(wout.T @ diffn + bout) ----
        attn = [persist.tile([128, L], F32R, tag=f"attn{j}", name=f"attn{j}") for j in range(4)]
        for j in range(4):
            for c in range(2):
                ps = ps_mm.tile([128, 512], F32, tag="mm", name="mm")
                for k in range(4):
                    nc.tensor.matmul(
                        ps,
                        wout_sb[k][:, 128 * j : 128 * (j + 1)],
                        diffn[k][:, 512 * c : 512 * (c + 1)],
                        start=(k == 0),
                        stop=(k == 3),
                    )
                nc.scalar.activation(
                    out=attn[j][:, 512 * c : 512 * (c + 1)], in_=ps,
                    func=AF.Silu, bias=bout[j],
                )

        if debug:
            for j in range(4):
                dma(out=dbg_attn_d[128 * j : 128 * (j + 1), :], in_=attn[j].bitcast(F32))

        # ---- final rms over CW=512 + residual + routing weight ----
        msqf = persist.tile([1, L], F32, tag="msqf", name="msqf")
        for c in range(2):
            fin = ps_sm.tile([1, 512], F32, tag="sm", name="sm")
            for j in range(4):
                asq = sqp.tile([128, 512], F32R, tag="sq", name="sq")
                nc.gpsimd.tensor_mul(
                    out=asq,
                    in0=attn[j][:, 512 * c : 512 * (c + 1)],
                    in1=attn[j][:, 512 * c : 512 * (c + 1)],
                )
                nc.tensor.matmul(fin, ones, asq, start=(j == 0), stop=(j == 3))
            nc.vector.tensor_copy(out=msqf[:, 512 * c : 512 * (c + 1)], in_=fin)
        rf = persist.tile([1, L], F32R, tag="rf", name="rf")
        nc.scalar.activation(
            out=rf, in_=msqf, func=AF.Abs_reciprocal_sqrt,
            scale=1.0 / 512.0, bias=eps_sb[0:1, :],
        )
        for j in range(4):
            nc.vector.tensor_scalar_mul(out=hT[j], in0=hT[j], scalar1=wrt)
        for c in range(2):
            rfbc = ps_mm.tile([128, 512], F32, tag="mm", name="mm")
            nc.tensor.matmul(
                rfbc, o1128, rf[:, 512 * c : 512 * (c + 1)],
                start=True, stop=True,
            )
            for j in range(4):
                tmp = tmpp.tile([128, 512], F32, tag="tmp", name="tmp")
                nc.vector.tensor_mul(
                    out=tmp, in0=attn[j][:, 512 * c : 512 * (c + 1)], in1=rfbc
                )
                y = yp.tile([128, 512], F32, tag="y", name="y")
                nc.vector.scalar_tensor_tensor(
                    out=y,
                    in0=tmp,
                    scalar=wnw[j],
                    in1=hT[j][:, 512 * c : 512 * (c + 1)],
                    op0=OP.mult,
                    op1=OP.add,
                )
                dma(out=yT_d[128 * j : 128 * (j + 1), 512 * c : 512 * (c + 1)], in_=y)
        est.close()

    nc.compile()
    return nc


def kernel(x, routing_weights, Wqkv, bqkv, Wout, bout, lq1, lk1, lq2, lk2, wq, wk, wh, wn):
    if "nc" not in _cache:
        _cache["nc"] = _build()
    nc = _cache["nc"]

    x = np.asarray(x, np.float32)
    routing_weights = np.asarray(routing_weights, np.float32)

    tri = np.triu(np.ones((128, 128), np.float32)).astype(ml_dtypes.bfloat16)
    e4 = np.zeros((128, 4), np.float32)
    for g in range(4):
        e4[32 * g : 32 * (g + 1), g] = 1.0
    b4 = e4.T.copy()
    b2c = np.zeros((33, 128), np.float32)
    b2c[0, 0:64] = 1.0
    b2c[32, 64:128] = 1.0
    ones = np.ones((128, 1), np.float32)
    o164 = np.ones((1, 64), np.float32)
    o1128 = np.ones((1, 128), np.float32)

    in_maps = []
    for i in range(8):
        b, n = i // 4, i % 4
        w = float(routing_weights[b, n])
        lam = float(
            np.exp(np.dot(lq1[n], lk1[n]).astype(np.float32))
            - np.exp(np.dot(lq2[n], lk2[n]).astype(np.float32))
            + np.float32(LAM0)
        )
        wqwk = (wq[n] * wk[n]).astype(np.float32)  # [32]
        in_maps.append(
            dict(
                hT=np.ascontiguousarray(x[b, :, CW * n : CW * (n + 1)].T),
                wqk=np.ascontiguousarray(Wqkv[n][:, : 2 * CW]),
                wv=np.ascontiguousarray(Wqkv[n][:, 2 * CW :]),
                wout=np.ascontiguousarray(Wout[n]),
                bqk=np.ascontiguousarray(bqkv[n][: 2 * CW].reshape(8, 128, 1)),
                bv=np.ascontiguousarray(bqkv[n][2 * CW :]),
                bout=np.ascontiguousarray(bout[n].reshape(4, 128, 1)),
                qmul=np.ascontiguousarray(np.tile(wqwk, 16).reshape(4, 128, 1)),
                whs=np.ascontiguousarray((np.tile(wh[n], 2) * 0.8).reshape(128, 1)).astype(np.float32),
                wnw=np.ascontiguousarray((wn[n] * w).reshape(4, 128, 1)).astype(np.float32),
                wrt=np.full((128, 1), w, np.float32),
                lam=np.full((1, 1), lam, np.float32),
                tri=tri,
                e4=e4,
                b4=b4,
                b2c=b2c,
                ones=ones,
                o164=o164,
                o1128=o1128,
            )
        )

    prof_dir = os.environ.get("KERNEL_PROFILE_DIR")
    if prof_dir:
        res = run_bass_kernel_spmd(
            nc, in_maps, list(range(8)), trace=True, tmpdir=prof_dir
        )
        _cache["exec_time_ns"] = res.exec_time_ns
    else:
        res = run_bass_kernel_spmd(nc, in_maps, list(range(8)))

    out = np.empty((B, L, N_CH * CW), np.float32)
    for i in range(8):
        b, n = i // 4, i % 4
        out[b, :, CW * n : CW * (n + 1)] = res.results[i]["yT"].T
    return out
